# revision 10
# baseline (speedup 1.0000x reference)
"""Trainium2 Bass kernel for nn_CompositionalNN_17308718202922.

Math: the reference only uses timestep 0 of both LSTM directions
(yf[0], yr[0]), and both directions consume the same first input
x[-1].  So the whole 64-step recurrence collapses to a single LSTM
cell step per direction plus the final linear+tanh:

    g_d   = [nt, h0_d] @ [Wih_d, Whh_d].T + bih_d + bhh_d      (d in {f, r})
    h_d   = sigmoid(o) * tanh(sigmoid(f) * c0_d + sigmoid(i) * tanh(g))
    out   = tanh([h_f, h_r] @ W_lin.T + b_lin)

Sharding (8 cores): tensor-parallel over the hidden dim.  Core k
computes hidden units [256k, 256k+256) of both directions (it holds
the 4*256 gate rows of each weight matrix), all-gathers the 512-long
h contribution, then computes output slice [256k, 256k+256) with its
(4096, 256) slice of the (permuted) W_lin.T.  Weights are shipped in
bf16 (halves the HBM traffic; matmuls accumulate fp32 in PSUM).
"""

import numpy as np
import ml_dtypes

import concourse.tile as tile
from concourse import bacc, mybir
from concourse.bass_utils import run_bass_kernel_spmd

H = 2048
NC = 8
HS = H // NC          # 256 hidden units per core
K1 = 2 * H            # 4096 contraction dim for gates ([x, h])
G = 4 * HS            # 1024 gate rows per core, order [i, f, o, g]
NK = K1 // 128        # 32 k-chunks per weight matrix
BF = ml_dtypes.bfloat16

_CACHE: dict = {}


def _build_program(variant: str = "full", wbufs: int = 4, dma_split: int = 1,
                   cpg: int = 4):
    f32 = mybir.dt.float32
    bf = mybir.dt.bfloat16
    nc = bacc.Bacc("TRN2", target_bir_lowering=False, debug=False, num_devices=NC)

    xh = nc.dram_tensor("xh", [128, 2 * NK, 2], bf, kind="ExternalInput")
    wf = nc.dram_tensor("wf", [K1, G], bf, kind="ExternalInput")
    wr = nc.dram_tensor("wr", [K1, G], bf, kind="ExternalInput")
    bg = nc.dram_tensor("bg", [2, G], f32, kind="ExternalInput")
    c0 = nc.dram_tensor("c0", [2, HS], f32, kind="ExternalInput")
    wl = nc.dram_tensor("wl", [K1, HS], bf, kind="ExternalInput")
    bl = nc.dram_tensor("bl", [1, HS], f32, kind="ExternalInput")
    if variant == "gates":
        out = nc.dram_tensor("out", [2, HS], f32, kind="ExternalOutput")
    else:
        out = nc.dram_tensor("out", [1, HS], f32, kind="ExternalOutput")

    SIG = mybir.ActivationFunctionType.Sigmoid
    TANH = mybir.ActivationFunctionType.Tanh

    with tile.TileContext(nc) as tc:
        with (
            tc.tile_pool(name="wpool", bufs=wbufs) as wpool,
            tc.tile_pool(name="wlpool", bufs=1) as wlpool,
            tc.tile_pool(name="const", bufs=1) as const,
            tc.tile_pool(name="work", bufs=1) as work,
            tc.tile_pool(name="psum", bufs=1, space="PSUM") as psum,
            tc.tile_pool(name="dram", bufs=1, space="DRAM") as dram,
        ):
            # --- constants / small inputs ---
            xh_t = const.tile([128, 2 * NK, 2], bf)
            nc.sync.dma_start(out=xh_t[:], in_=xh.ap())
            bg_t = const.tile([2, G], f32)
            nc.sync.dma_start(out=bg_t[:], in_=bg.ap())
            c0_t = const.tile([2, HS], f32)
            nc.sync.dma_start(out=c0_t[:], in_=c0.ap())
            bl_t = const.tile([1, HS], f32)
            nc.sync.dma_start(out=bl_t[:], in_=bl.ap())

            if variant != "gates":
                # W_lin slice, fully resident: (128, 32, 256) bf16 = 2 MiB
                wl_t = wlpool.tile([128, NK, HS], bf)
                nc.sync.dma_start(
                    out=wl_t[:], in_=wl.ap().rearrange("(c p) n -> p c n", p=128)
                )

            # --- gate matvecs: 64 k-chunks (32 fwd + 32 rev) ---
            # psum_g0 rows [fwd, rev] x cols [i(256) | f(256)]
            # psum_g1 rows [fwd, rev] x cols [o(256) | g(256)]
            psum_g0 = psum.tile([2, 512], f32)
            psum_g1 = psum.tile([2, 512], f32)
            CPG = cpg  # k-chunks per DMA'd weight tile
            for a in range(2 * NK // CPG):
                w = wf if a < NK // CPG else wr
                aa = a % (NK // CPG)
                wt = wpool.tile([128, CPG, G], bf)
                src_ap = w.ap().rearrange("(a c p) n -> a p c n", p=128, c=CPG)[aa]
                if dma_split == 1:
                    nc.sync.dma_start(out=wt[:], in_=src_ap)
                else:
                    step = CPG // dma_split
                    for si in range(dma_split):
                        lo, hi = si * step, (si + 1) * step
                        nc.sync.dma_start(
                            out=wt[:, lo:hi, :], in_=src_ap[:, lo:hi, :]
                        )
                for cc in range(CPG):
                    c = a * CPG + cc
                    lhsT = xh_t[:, c, :]            # (128, 2)
                    nc.tensor.matmul(
                        psum_g0[:], lhsT, wt[:, cc, 0:512],
                        start=(c == 0), stop=(c == 2 * NK - 1),
                    )
                    nc.tensor.matmul(
                        psum_g1[:], lhsT, wt[:, cc, 512:1024],
                        start=(c == 0), stop=(c == 2 * NK - 1),
                    )

            # --- bias add + LSTM cell elementwise (both dirs on 2 partitions) ---
            g0 = work.tile([2, 512], f32)
            nc.vector.tensor_add(g0[:], psum_g0[:], bg_t[:, 0:512])
            g1 = work.tile([2, 512], f32)
            nc.vector.tensor_add(g1[:], psum_g1[:], bg_t[:, 512:1024])
            s = work.tile([2, 768], f32)            # [sig(i) | sig(f) | sig(o)]
            gt = work.tile([2, HS], f32)            # tanh(g)
            nc.scalar.activation(s[:, 0:512], g0[:], SIG)
            nc.scalar.activation(s[:, 512:768], g1[:, 0:256], SIG)
            nc.scalar.activation(gt[:], g1[:, 256:512], TANH)
            t1 = work.tile([2, HS], f32)
            nc.vector.tensor_mul(t1[:], s[:, 0:256], gt[:])         # i * g
            t2 = work.tile([2, HS], f32)
            nc.vector.tensor_mul(t2[:], s[:, 256:512], c0_t[:])     # f * c0
            cnew = work.tile([2, HS], f32)
            nc.vector.tensor_add(cnew[:], t1[:], t2[:])
            ct = work.tile([2, HS], f32)
            nc.scalar.activation(ct[:], cnew[:], TANH)

            if variant == "gates":
                hh32 = work.tile([2, HS], f32)
                nc.vector.tensor_mul(hh32[:], s[:, 512:768], ct[:])
                nc.sync.dma_start(out=out.ap(), in_=hh32[:])
            else:
                hh = work.tile([2, HS], bf)
                nc.vector.tensor_mul(hh[:], s[:, 512:768], ct[:])   # o * tanh(c)

                # --- all-gather h contributions (1 KiB) ---
                bounce = dram.tile([2, HS], bf)
                agout = dram.tile([4 * NC, 128], bf)    # (32, 128)
                nc.sync.dma_start(out=bounce[:], in_=hh[:])
                if variant == "timing":
                    # collective-free twin for TimelineSim cost modelling
                    nc.sync.dma_start(out=agout[0:4, :], in_=bounce[:])
                else:
                    nc.gpsimd.collective_compute(
                        "AllGather",
                        mybir.AluOpType.bypass,
                        replica_groups=[list(range(NC))],
                        ins=[bounce.opt()],
                        outs=[agout.opt()],
                    )
                # combined vector as k-chunk lhsT columns: hT[p, c] = ag[c, p]
                hT = const.tile([128, NK], bf)
                nc.sync.dma_start(
                    out=hT[:], in_=agout[:].rearrange("c p -> p c")
                )

                # --- final linear slice + bias + tanh ---
                psum_o = psum.tile([1, HS], f32)
                for c in range(NK):
                    nc.tensor.matmul(
                        psum_o[:], hT[:, c:c + 1], wl_t[:, c, :],
                        start=(c == 0), stop=(c == NK - 1),
                    )
                ob = work.tile([1, HS], f32)
                nc.vector.tensor_add(ob[:], psum_o[:], bl_t[:])
                out_sb = work.tile([1, HS], f32)
                nc.scalar.activation(out_sb[:], ob[:], TANH)
                nc.sync.dma_start(out=out.ap(), in_=out_sb[:])

    nc.compile()
    return nc


def _prep_in_maps(x, h0_fwd, c0_fwd, h0_rev, c0_rev,
                  Wih_f, Whh_f, bih_f, bhh_f,
                  Wih_r, Whh_r, bih_r, bhh_r,
                  W_lin, b_lin):
    f32 = np.float32
    nt = np.asarray(x, f32)[-1, 0]                               # (2048,)
    xh_f = np.concatenate([nt, np.asarray(h0_fwd, f32)[0]])      # (4096,)
    xh_r = np.concatenate([nt, np.asarray(h0_rev, f32)[0]])
    Wf_cat = np.concatenate(
        [np.asarray(Wih_f, f32), np.asarray(Whh_f, f32)], axis=1)  # (8192, 4096)
    Wr_cat = np.concatenate(
        [np.asarray(Wih_r, f32), np.asarray(Whh_r, f32)], axis=1)
    bf_full = np.asarray(bih_f, f32) + np.asarray(bhh_f, f32)
    br_full = np.asarray(bih_r, f32) + np.asarray(bhh_r, f32)
    WT = np.asarray(W_lin, f32).T                                # (4096, 2048)
    b_lin = np.asarray(b_lin, f32)

    # xh lhsT columns: chunk c < 32 -> fwd (col 0), c >= 32 -> rev (col 1)
    xh_arr = np.zeros((128, 2 * NK, 2), BF)
    xh_arr[:, :NK, 0] = xh_f.astype(BF).reshape(NK, 128).T
    xh_arr[:, NK:, 1] = xh_r.astype(BF).reshape(NK, 128).T

    # gathered combined layout: rank k contributes [h_f_k, h_r_k]
    orig_idx = np.empty(2 * H, np.int64)
    t = np.arange(HS)
    for k in range(NC):
        orig_idx[2 * HS * k: 2 * HS * k + HS] = k * HS + t
        orig_idx[2 * HS * k + HS: 2 * HS * (k + 1)] = H + k * HS + t
    WLP = np.ascontiguousarray(WT[orig_idx]).astype(BF)          # (4096, 2048)

    in_maps = []
    for k in range(NC):
        sl = np.arange(k * HS, (k + 1) * HS)
        rowsel = np.concatenate([g * H + sl for g in (0, 1, 3, 2)])  # i, f, o, g
        in_maps.append({
            "xh": xh_arr,
            "wf": np.ascontiguousarray(Wf_cat[rowsel].T).astype(BF),
            "wr": np.ascontiguousarray(Wr_cat[rowsel].T).astype(BF),
            "bg": np.stack([bf_full[rowsel], br_full[rowsel]]),
            "c0": np.stack([np.asarray(c0_fwd, f32)[0, sl],
                            np.asarray(c0_rev, f32)[0, sl]]),
            "wl": np.ascontiguousarray(WLP[:, sl]),
            "bl": b_lin[sl][None, :],
        })
    return in_maps


def kernel(**inputs) -> np.ndarray:
    if "nc" not in _CACHE:
        _CACHE["nc"] = _build_program("full")
    nc = _CACHE["nc"]
    in_maps = _prep_in_maps(**inputs)
    res = run_bass_kernel_spmd(nc, in_maps, core_ids=list(range(NC)))
    return np.concatenate(
        [res.results[k]["out"][0] for k in range(NC)]
    )[None, :].astype(np.float32)


# revision 11
# speedup vs baseline: 1.0107x; 1.0107x over previous
"""Trainium2 Bass kernel for nn_CompositionalNN_17308718202922.

Math: the reference only uses timestep 0 of both LSTM directions
(yf[0], yr[0]), and both directions consume the same first input
x[-1].  So the whole 64-step recurrence collapses to a single LSTM
cell step per direction plus the final linear+tanh:

    g_d   = [nt, h0_d] @ [Wih_d, Whh_d].T + bih_d + bhh_d      (d in {f, r})
    h_d   = sigmoid(o) * tanh(sigmoid(f) * c0_d + sigmoid(i) * tanh(g))
    out   = tanh([h_f, h_r] @ W_lin.T + b_lin)

Sharding (8 cores): tensor-parallel over the hidden dim.  Core k
computes hidden units [256k, 256k+256) of both directions (it holds
the 4*256 gate rows of each weight matrix), all-gathers the 512-long
h contribution, then computes output slice [256k, 256k+256) with its
(4096, 256) slice of the (permuted) W_lin.T.  Weights are shipped in
bf16 (halves the HBM traffic; matmuls accumulate fp32 in PSUM).
"""

import numpy as np
import ml_dtypes

import concourse.tile as tile
from concourse import bacc, mybir
from concourse.bass_utils import run_bass_kernel_spmd

H = 2048
NC = 8
HS = H // NC          # 256 hidden units per core
K1 = 2 * H            # 4096 contraction dim for gates ([x, h])
G = 4 * HS            # 1024 gate rows per core, order [i, f, o, g]
NK = K1 // 128        # 32 k-chunks per weight matrix
BF = ml_dtypes.bfloat16

_CACHE: dict = {}


def _build_program(variant: str = "full", wbufs: int = 8, dma_split: int = 2,
                   cpg: int = 4):
    f32 = mybir.dt.float32
    bf = mybir.dt.bfloat16
    nc = bacc.Bacc("TRN2", target_bir_lowering=False, debug=False, num_devices=NC)

    xh = nc.dram_tensor("xh", [128, 2 * NK, 2], bf, kind="ExternalInput")
    wf = nc.dram_tensor("wf", [K1, G], bf, kind="ExternalInput")
    wr = nc.dram_tensor("wr", [K1, G], bf, kind="ExternalInput")
    bg = nc.dram_tensor("bg", [2, G], f32, kind="ExternalInput")
    c0 = nc.dram_tensor("c0", [2, HS], f32, kind="ExternalInput")
    wl = nc.dram_tensor("wl", [K1, HS], bf, kind="ExternalInput")
    bl = nc.dram_tensor("bl", [1, HS], f32, kind="ExternalInput")
    if variant == "gates":
        out = nc.dram_tensor("out", [2, HS], f32, kind="ExternalOutput")
    else:
        out = nc.dram_tensor("out", [1, HS], f32, kind="ExternalOutput")

    SIG = mybir.ActivationFunctionType.Sigmoid
    TANH = mybir.ActivationFunctionType.Tanh

    with tile.TileContext(nc) as tc:
        with (
            tc.tile_pool(name="wpool", bufs=wbufs) as wpool,
            tc.tile_pool(name="wlpool", bufs=1) as wlpool,
            tc.tile_pool(name="const", bufs=1) as const,
            tc.tile_pool(name="work", bufs=1) as work,
            tc.tile_pool(name="psum", bufs=1, space="PSUM") as psum,
            tc.tile_pool(name="dram", bufs=1, space="DRAM") as dram,
        ):
            # --- constants / small inputs ---
            xh_t = const.tile([128, 2 * NK, 2], bf)
            nc.sync.dma_start(out=xh_t[:], in_=xh.ap())
            bg_t = const.tile([2, G], f32)
            nc.sync.dma_start(out=bg_t[:], in_=bg.ap())
            c0_t = const.tile([2, HS], f32)
            nc.sync.dma_start(out=c0_t[:], in_=c0.ap())
            bl_t = const.tile([1, HS], f32)
            nc.sync.dma_start(out=bl_t[:], in_=bl.ap())

            if variant != "gates":
                # W_lin slice, fully resident: (128, 32, 256) bf16 = 2 MiB
                wl_t = wlpool.tile([128, NK, HS], bf)
                nc.sync.dma_start(
                    out=wl_t[:], in_=wl.ap().rearrange("(c p) n -> p c n", p=128)
                )

            # --- gate matvecs: 64 k-chunks (32 fwd + 32 rev) ---
            # psum_g0 rows [fwd, rev] x cols [i(256) | f(256)]
            # psum_g1 rows [fwd, rev] x cols [o(256) | g(256)]
            psum_g0 = psum.tile([2, 512], f32)
            psum_g1 = psum.tile([2, 512], f32)
            CPG = cpg  # k-chunks per DMA'd weight tile
            for a in range(2 * NK // CPG):
                w = wf if a < NK // CPG else wr
                aa = a % (NK // CPG)
                wt = wpool.tile([128, CPG, G], bf)
                src_ap = w.ap().rearrange("(a c p) n -> a p c n", p=128, c=CPG)[aa]
                if dma_split == 1:
                    nc.sync.dma_start(out=wt[:], in_=src_ap)
                else:
                    step = CPG // dma_split
                    for si in range(dma_split):
                        lo, hi = si * step, (si + 1) * step
                        nc.sync.dma_start(
                            out=wt[:, lo:hi, :], in_=src_ap[:, lo:hi, :]
                        )
                for cc in range(CPG):
                    c = a * CPG + cc
                    lhsT = xh_t[:, c, :]            # (128, 2)
                    nc.tensor.matmul(
                        psum_g0[:], lhsT, wt[:, cc, 0:512],
                        start=(c == 0), stop=(c == 2 * NK - 1),
                    )
                    nc.tensor.matmul(
                        psum_g1[:], lhsT, wt[:, cc, 512:1024],
                        start=(c == 0), stop=(c == 2 * NK - 1),
                    )

            # --- bias add + LSTM cell elementwise (both dirs on 2 partitions) ---
            g0 = work.tile([2, 512], f32)
            nc.vector.tensor_add(g0[:], psum_g0[:], bg_t[:, 0:512])
            g1 = work.tile([2, 512], f32)
            nc.vector.tensor_add(g1[:], psum_g1[:], bg_t[:, 512:1024])
            s = work.tile([2, 768], f32)            # [sig(i) | sig(f) | sig(o)]
            gt = work.tile([2, HS], f32)            # tanh(g)
            nc.scalar.activation(s[:, 0:512], g0[:], SIG)
            nc.scalar.activation(s[:, 512:768], g1[:, 0:256], SIG)
            nc.scalar.activation(gt[:], g1[:, 256:512], TANH)
            t1 = work.tile([2, HS], f32)
            nc.vector.tensor_mul(t1[:], s[:, 0:256], gt[:])         # i * g
            t2 = work.tile([2, HS], f32)
            nc.vector.tensor_mul(t2[:], s[:, 256:512], c0_t[:])     # f * c0
            cnew = work.tile([2, HS], f32)
            nc.vector.tensor_add(cnew[:], t1[:], t2[:])
            ct = work.tile([2, HS], f32)
            nc.scalar.activation(ct[:], cnew[:], TANH)

            if variant == "gates":
                hh32 = work.tile([2, HS], f32)
                nc.vector.tensor_mul(hh32[:], s[:, 512:768], ct[:])
                nc.sync.dma_start(out=out.ap(), in_=hh32[:])
            else:
                hh = work.tile([2, HS], bf)
                nc.vector.tensor_mul(hh[:], s[:, 512:768], ct[:])   # o * tanh(c)

                # --- all-gather h contributions (1 KiB) ---
                bounce = dram.tile([2, HS], bf)
                agout = dram.tile([4 * NC, 128], bf)    # (32, 128)
                nc.sync.dma_start(out=bounce[:], in_=hh[:])
                if variant == "timing":
                    # collective-free twin for TimelineSim cost modelling
                    nc.sync.dma_start(out=agout[0:4, :], in_=bounce[:])
                else:
                    nc.gpsimd.collective_compute(
                        "AllGather",
                        mybir.AluOpType.bypass,
                        replica_groups=[list(range(NC))],
                        ins=[bounce.opt()],
                        outs=[agout.opt()],
                    )
                # combined vector as k-chunk lhsT columns: hT[p, c] = ag[c, p]
                hT = const.tile([128, NK], bf)
                nc.sync.dma_start(
                    out=hT[:], in_=agout[:].rearrange("c p -> p c")
                )

                # --- final linear slice + bias + tanh ---
                psum_o = psum.tile([1, HS], f32)
                for c in range(NK):
                    nc.tensor.matmul(
                        psum_o[:], hT[:, c:c + 1], wl_t[:, c, :],
                        start=(c == 0), stop=(c == NK - 1),
                    )
                ob = work.tile([1, HS], f32)
                nc.vector.tensor_add(ob[:], psum_o[:], bl_t[:])
                out_sb = work.tile([1, HS], f32)
                nc.scalar.activation(out_sb[:], ob[:], TANH)
                nc.sync.dma_start(out=out.ap(), in_=out_sb[:])

    nc.compile()
    return nc


def _prep_in_maps(x, h0_fwd, c0_fwd, h0_rev, c0_rev,
                  Wih_f, Whh_f, bih_f, bhh_f,
                  Wih_r, Whh_r, bih_r, bhh_r,
                  W_lin, b_lin):
    f32 = np.float32
    nt = np.asarray(x, f32)[-1, 0]                               # (2048,)
    xh_f = np.concatenate([nt, np.asarray(h0_fwd, f32)[0]])      # (4096,)
    xh_r = np.concatenate([nt, np.asarray(h0_rev, f32)[0]])
    Wf_cat = np.concatenate(
        [np.asarray(Wih_f, f32), np.asarray(Whh_f, f32)], axis=1)  # (8192, 4096)
    Wr_cat = np.concatenate(
        [np.asarray(Wih_r, f32), np.asarray(Whh_r, f32)], axis=1)
    bf_full = np.asarray(bih_f, f32) + np.asarray(bhh_f, f32)
    br_full = np.asarray(bih_r, f32) + np.asarray(bhh_r, f32)
    WT = np.asarray(W_lin, f32).T                                # (4096, 2048)
    b_lin = np.asarray(b_lin, f32)

    # xh lhsT columns: chunk c < 32 -> fwd (col 0), c >= 32 -> rev (col 1)
    xh_arr = np.zeros((128, 2 * NK, 2), BF)
    xh_arr[:, :NK, 0] = xh_f.astype(BF).reshape(NK, 128).T
    xh_arr[:, NK:, 1] = xh_r.astype(BF).reshape(NK, 128).T

    # gathered combined layout: rank k contributes [h_f_k, h_r_k]
    orig_idx = np.empty(2 * H, np.int64)
    t = np.arange(HS)
    for k in range(NC):
        orig_idx[2 * HS * k: 2 * HS * k + HS] = k * HS + t
        orig_idx[2 * HS * k + HS: 2 * HS * (k + 1)] = H + k * HS + t
    WLP = np.ascontiguousarray(WT[orig_idx]).astype(BF)          # (4096, 2048)

    in_maps = []
    for k in range(NC):
        sl = np.arange(k * HS, (k + 1) * HS)
        rowsel = np.concatenate([g * H + sl for g in (0, 1, 3, 2)])  # i, f, o, g
        in_maps.append({
            "xh": xh_arr,
            "wf": np.ascontiguousarray(Wf_cat[rowsel].T).astype(BF),
            "wr": np.ascontiguousarray(Wr_cat[rowsel].T).astype(BF),
            "bg": np.stack([bf_full[rowsel], br_full[rowsel]]),
            "c0": np.stack([np.asarray(c0_fwd, f32)[0, sl],
                            np.asarray(c0_rev, f32)[0, sl]]),
            "wl": np.ascontiguousarray(WLP[:, sl]),
            "bl": b_lin[sl][None, :],
        })
    return in_maps


def kernel(**inputs) -> np.ndarray:
    if "nc" not in _CACHE:
        _CACHE["nc"] = _build_program("full")
    nc = _CACHE["nc"]
    in_maps = _prep_in_maps(**inputs)
    res = run_bass_kernel_spmd(nc, in_maps, core_ids=list(range(NC)))
    return np.concatenate(
        [res.results[k]["out"][0] for k in range(NC)]
    )[None, :].astype(np.float32)
